# revision 12
# baseline (speedup 1.0000x reference)
"""AdaAggLayer Trainium2 kernel.

Data-parallel over batch: 8 NeuronCores x 4 samples each. Per core:
  - attention (global avg pool -> 1x1 -> relu -> 1x1 -> sigmoid) on PE/ACT/DVE
  - align transform w_alT[e] = (align[e] @ w[e]) stored transposed [i,o] on PE
  - per-sample weight aggregation sum_e att[b,e]*w_alT[e] on DVE (bf16)
  - per-sample 3x3 conv as 9 shifted matmuls accumulating in PSUM (bf16)
  - bias epilogue fused into the PSUM->SBUF copy on ACT
No collectives: inputs are sharded/replicated host-side, outputs concatenated.
"""

import contextlib
import importlib.util
import os
import sys
import types

sys.path.insert(0, "/opt/trn_rl_repo")

import numpy as np
import ml_dtypes

import concourse.bass as bass
import concourse.mybir as mybir
import concourse.tile as tile
from concourse import bacc
from concourse.bass_utils import run_bass_kernel_spmd

N_CORES = 8
B, I, O, E, HID = 32, 256, 256, 5, 65
H = W = 56
HP = H + 2  # zero-padded spatial
BL = B // N_CORES  # samples per core
KK = 9  # 3x3 taps
NBLK = 7  # row blocks of 8 output rows
RB = 8  # rows per block
BF16 = mybir.dt.bfloat16
F32 = mybir.dt.float32

_NC_CACHE = None


def _install_ntff_hook():
    """Register the axon NTFF profiling hook (the image's antenv lacks it)."""
    if "antenv.axon_hooks" in sys.modules:
        return
    try:
        spec = importlib.util.spec_from_file_location(
            "trn_boot", "/root/.axon_site/trn_agent_boot/trn_boot.py"
        )
        tb = importlib.util.module_from_spec(spec)
        spec.loader.exec_module(tb)
        hook = tb._ntff_profile_via_ctypes("/opt/axon/libaxon_pjrt.so")
    except Exception:
        hook = None
    mod = types.ModuleType("antenv.axon_hooks")
    mod.get_axon_ntff_profile_hook = lambda: hook
    sys.modules["antenv.axon_hooks"] = mod


def _emit(nc, tc, ctx):
    x_d = nc.dram_tensor("x", [BL, I, HP, HP], BF16, kind="ExternalInput")
    w_d = nc.dram_tensor("w", [E, O, I * KK], BF16, kind="ExternalInput")
    at_d = nc.dram_tensor("alignT", [E, O, O], BF16, kind="ExternalInput")
    w1_d = nc.dram_tensor("w1T", [I, HID], F32, kind="ExternalInput")
    w2_d = nc.dram_tensor("w2T", [HID, E], F32, kind="ExternalInput")
    b2_d = nc.dram_tensor("b2", [E, 1], F32, kind="ExternalInput")
    bias_d = nc.dram_tensor("bias", [E, O], F32, kind="ExternalInput")
    out_d = nc.dram_tensor("out", [BL, O, H, W], F32, kind="ExternalOutput")

    const = ctx.enter_context(tc.tile_pool(name="const", bufs=1))
    wstream = ctx.enter_context(tc.tile_pool(name="wstream", bufs=6))
    xpool = ctx.enter_context(tc.tile_pool(name="x", bufs=1))
    aggp = ctx.enter_context(tc.tile_pool(name="agg", bufs=BL))
    tmpp = ctx.enter_context(tc.tile_pool(name="tmp", bufs=2))
    stagep = ctx.enter_context(tc.tile_pool(name="stage", bufs=4))
    t_psum = ctx.enter_context(tc.tile_pool(name="tps", bufs=3, space="PSUM"))
    s_psum = ctx.enter_context(tc.tile_pool(name="sps", bufs=1, space="PSUM"))
    c_psum = ctx.enter_context(tc.tile_pool(name="cps", bufs=4, space="PSUM"))
    NG = 3  # kk chunks per aggregation group (3 groups of 3)

    # ---- constants in ----
    at_sb = const.tile([128, E, 2, O], BF16)  # part = o_old % 128
    w1_sb = const.tile([128, 2, HID], F32)  # part = i % 128
    w2_sb = const.tile([HID, E], F32)
    b2_sb = const.tile([E, 1], F32)
    bias_sb = const.tile([E, O], F32)
    ones_sb = const.tile([1, 128], F32)
    walT = const.tile([128, E, KK, 2, O], BF16)  # part = i % 128 (per i-half)
    pooledT = const.tile([128, 2, BL], F32)  # part = i % 128
    h_sb = const.tile([HID, BL], F32)
    att_sb = const.tile([E, BL], F32)
    att_row = const.tile([1, BL * E], F32)
    att_bc = const.tile([128, BL, E], F32)
    aggb_sb = const.tile([128, 2, BL], F32)  # part = o % 128

    # ---- DMA schedule: x0 first (attention-b0 head), then weights, then x1-3
    x_sb = {}

    def dma_x(b):
        for ih in range(2):
            t = xpool.tile([128, HP, HP], BF16, tag=f"x{b}_{ih}")
            nc.sync.dma_start(out=t[:, :, :], in_=x_d[b, ih * 128 : (ih + 1) * 128, :, :])
            x_sb[(b, ih)] = t
            # pooled sum via tensor_scalar accum_out (4x bf16 mode); the main
            # output is an in-place x*1.0 no-op.
            nc.vector.tensor_scalar(
                out=t[:, :, :],
                in0=t[:, :, :],
                scalar1=1.0,
                scalar2=None,
                op0=mybir.AluOpType.mult,
                op1=mybir.AluOpType.add,
                accum_out=pooledT[:, ih, b : b + 1],
            )

    def attention(b):
        hp = s_psum.tile([HID, 1], F32, tag="sps")
        for ih in range(2):
            nc.tensor.matmul(
                hp[:, :],
                lhsT=w1_sb[:, ih, :],
                rhs=pooledT[:, ih, b : b + 1],
                start=(ih == 0),
                stop=(ih == 1),
            )
        nc.scalar.activation(
            h_sb[:, b : b + 1], hp[:, :], mybir.ActivationFunctionType.Relu
        )
        ap = s_psum.tile([E, 1], F32, tag="sps")
        nc.tensor.matmul(ap[:, :], lhsT=w2_sb[:, :], rhs=h_sb[:, b : b + 1])
        nc.scalar.activation(
            att_sb[:, b : b + 1],
            ap[:, :],
            mybir.ActivationFunctionType.Sigmoid,
            bias=b2_sb[:, 0:1],
        )
        # gather the 5 att values onto partition 0, then broadcast to all 128
        nc.sync.dma_start(out=att_row[0:1, b * E : (b + 1) * E], in_=att_sb[:, b : b + 1])
        bp = s_psum.tile([128, E], F32, tag="sps")
        nc.tensor.matmul(
            bp[:, :], lhsT=ones_sb[0:1, :], rhs=att_row[0:1, b * E : (b + 1) * E]
        )
        nc.vector.tensor_copy(out=att_bc[:, b, :], in_=bp[:, :])
        # aggregated bias agg_b[o, b] = sum_e att[e,b] * bias[e, o]
        for ot in range(2):
            gp = s_psum.tile([128, 1], F32, tag="sps")
            nc.tensor.matmul(
                gp[:, :],
                lhsT=bias_sb[:, ot * 128 : (ot + 1) * 128],
                rhs=att_sb[:, b : b + 1],
            )
            nc.vector.tensor_copy(out=aggb_sb[:, ot, b : b + 1], in_=gp[:, :])

    dma_x(0)
    for ih in range(2):
        nc.sync.dma_start(out=w1_sb[:, ih, :], in_=w1_d[ih * 128 : (ih + 1) * 128, :])
    nc.sync.dma_start(out=w2_sb[:, :], in_=w2_d[:, :])
    nc.sync.dma_start(out=b2_sb[:, :], in_=b2_d[:, :])
    nc.sync.dma_start(out=bias_sb[:, :], in_=bias_d[:, :])
    nc.vector.memset(ones_sb[:, :], 1.0)

    wt_all = {}
    for e in range(E):
        for oh in range(2):
            t = wstream.tile([128, I, KK], BF16, tag="wst")
            nc.sync.dma_start(out=t[:, :, :], in_=w_d[e, oh * 128 : (oh + 1) * 128, :])
            wt_all[(e, oh)] = t
            nc.sync.dma_start(
                out=at_sb[:, e, oh, :], in_=at_d[e, oh * 128 : (oh + 1) * 128, :]
            )
    for b in range(1, BL):
        dma_x(b)

    attention(0)

    # ---- align transform: walT[e][kk, ih][ii, o] = sum_o_old w[e,o_old,(i,kk)] alignT[e][o_old, o]
    # Full-bank PSUM groups: both ih halves of one (e,kk) in one [128,512] bank.
    for e in range(E):
        for kk in range(KK):
            tp = t_psum.tile([128, 2, O], F32, tag="tps")
            for ih in range(2):
                for oh in range(2):
                    nc.tensor.matmul(
                        tp[:, ih, :],
                        lhsT=wt_all[(e, oh)][:, ih * 128 : (ih + 1) * 128, kk],
                        rhs=at_sb[:, e, oh, :],
                        start=(oh == 0),
                        stop=(oh == 1),
                    )
            nc.vector.tensor_copy(out=walT[:, e, kk, :, :], in_=tp[:, :, :])
        # interleave remaining attention chains while the transform owns PE
        if e - 1 in range(1, BL):
            attention(e - 1)

    # ---- per-sample: aggregate weights (DVE, kk-chunked) then conv (PE) ----
    for b in range(BL):
        # scale+add tree: tensor_scalar (4x bf16) + tensor_tensor (2x bf16)
        # beats the 1x-mode fused scalar_tensor_tensor chain. Chunked by kk
        # group so the conv can start consuming before the full agg is done.
        aggs = []
        for g in range(NG):
            k0, k1 = g * 3, g * 3 + 3
            agg = aggp.tile([128, 3, 2, O], BF16, tag=f"agg{g}")
            nc.vector.tensor_scalar_mul(
                agg[:, :, :, :], walT[:, 0, k0:k1, :, :], att_bc[:, b, 0:1]
            )
            for e in range(1, E):
                tmp = tmpp.tile([128, 3, 2, O], BF16, tag="tmp")
                nc.vector.tensor_scalar_mul(
                    tmp[:, :, :, :], walT[:, e, k0:k1, :, :], att_bc[:, b, e : e + 1]
                )
                nc.vector.tensor_add(
                    out=agg[:, :, :, :], in0=agg[:, :, :, :], in1=tmp[:, :, :, :]
                )
            aggs.append(agg)

        for ot in range(2):
            for blk in range(NBLK):
                cp = c_psum.tile([128, RB, W], F32, tag="cps")
                r0 = blk * RB
                n_mm = 2 * KK
                k = 0
                for g in range(NG):
                    for kq in range(3):
                        kk = g * 3 + kq
                        di, dj = kk // 3 - 1, kk % 3 - 1
                        for ih in range(2):
                            nc.tensor.matmul(
                                cp[:, :, :],
                                lhsT=aggs[g][:, kq, ih, ot * 128 : (ot + 1) * 128],
                                rhs=x_sb[(b, ih)][
                                    :, r0 + di + 1 : r0 + di + 1 + RB, dj + 1 : dj + 1 + W
                                ],
                                start=(k == 0),
                                stop=(k == n_mm - 1),
                            )
                            k += 1
                st = stagep.tile([128, RB, W], F32, tag="stage")
                nc.scalar.activation(
                    st[:, :, :],
                    cp[:, :, :],
                    mybir.ActivationFunctionType.Identity,
                    bias=aggb_sb[:, ot, b : b + 1],
                )
                nc.sync.dma_start(
                    out=out_d[b, ot * 128 : (ot + 1) * 128, r0 : r0 + RB, :],
                    in_=st[:, :, :],
                )


def _build():
    nc = bacc.Bacc("TRN2", target_bir_lowering=False, debug=False, num_devices=N_CORES)
    with contextlib.ExitStack() as ctx:
        tc = ctx.enter_context(tile.TileContext(nc))
        _emit(nc, tc, ctx)
    nc.compile()
    return nc


def _get_nc():
    global _NC_CACHE
    if _NC_CACHE is None:
        _NC_CACHE = _build()
    return _NC_CACHE


def _run(trace=False, **inputs):
    x = np.asarray(inputs["x"], np.float32)
    weight = np.asarray(inputs["weight"], np.float32)
    bias = np.asarray(inputs["bias"], np.float32)
    align = np.asarray(inputs["align"], np.float32)
    w1 = np.asarray(inputs["attn_w1"], np.float32)
    w2 = np.asarray(inputs["attn_w2"], np.float32)
    b2 = np.asarray(inputs["attn_b2"], np.float32)

    xp = np.zeros((B, I, HP, HP), dtype=ml_dtypes.bfloat16)
    xp[:, :, 1 : 1 + H, 1 : 1 + W] = x
    w_bf = weight.reshape(E, O, I * KK).astype(ml_dtypes.bfloat16)
    alT = np.ascontiguousarray(align.transpose(0, 2, 1)).astype(ml_dtypes.bfloat16)
    w1T = np.ascontiguousarray((w1 / float(H * W)).T)
    w2T = np.ascontiguousarray(w2.T)
    b2c = np.ascontiguousarray(b2.reshape(E, 1))

    nc = _get_nc()
    in_maps = []
    for c in range(N_CORES):
        in_maps.append(
            {
                "x": xp[c * BL : (c + 1) * BL],
                "w": w_bf,
                "alignT": alT,
                "w1T": w1T,
                "w2T": w2T,
                "b2": b2c,
                "bias": bias,
            }
        )
    if trace:
        _install_ntff_hook()
    res = run_bass_kernel_spmd(
        nc, in_maps, core_ids=list(range(N_CORES)), trace=trace
    )
    out = np.concatenate([res.results[c]["out"] for c in range(N_CORES)], axis=0)
    return out, res


def kernel(**inputs):
    out, _ = _run(trace=False, **inputs)
    return out


def kernel_profiled(**inputs):
    out, res = _run(trace=True, **inputs)
    return out, res


# revision 16
# speedup vs baseline: 1.2107x; 1.2107x over previous
"""AdaAggLayer Trainium2 kernel.

Data-parallel over batch: 8 NeuronCores x 4 samples each. Per core:
  - attention (global avg pool -> 1x1 -> relu -> 1x1 -> sigmoid) on PE/ACT/DVE
  - align transform w_alT[e] = (align[e] @ w[e]) stored transposed [i,o] on PE
  - per-sample weight aggregation sum_e att[b,e]*w_alT[e] on DVE (bf16)
  - per-sample 3x3 conv as 9 shifted matmuls accumulating in PSUM (bf16)
  - bias epilogue fused into the PSUM->SBUF copy on ACT
No collectives: inputs are sharded/replicated host-side, outputs concatenated.
"""

import contextlib
import importlib.util
import os
import sys
import types

sys.path.insert(0, "/opt/trn_rl_repo")

import numpy as np
import ml_dtypes

import concourse.bass as bass
import concourse.mybir as mybir
import concourse.tile as tile
from concourse import bacc
from concourse.bass_utils import run_bass_kernel_spmd

N_CORES = 8
B, I, O, E, HID = 32, 256, 256, 5, 65
H = W = 56
HP = H + 2  # zero-padded spatial
BL = B // N_CORES  # samples per core
KK = 9  # 3x3 taps
NBLK = 7  # row blocks of 8 output rows
RB = 8  # rows per block
BF16 = mybir.dt.bfloat16
F32 = mybir.dt.float32

_NC_CACHE = None


def _install_ntff_hook():
    """Register the axon NTFF profiling hook (the image's antenv lacks it)."""
    if "antenv.axon_hooks" in sys.modules:
        return
    try:
        spec = importlib.util.spec_from_file_location(
            "trn_boot", "/root/.axon_site/trn_agent_boot/trn_boot.py"
        )
        tb = importlib.util.module_from_spec(spec)
        spec.loader.exec_module(tb)
        hook = tb._ntff_profile_via_ctypes("/opt/axon/libaxon_pjrt.so")
    except Exception:
        hook = None
    mod = types.ModuleType("antenv.axon_hooks")
    mod.get_axon_ntff_profile_hook = lambda: hook
    sys.modules["antenv.axon_hooks"] = mod


def _emit(nc, tc, ctx):
    x_d = nc.dram_tensor("x", [BL, I, HP, HP], BF16, kind="ExternalInput")
    w_d = nc.dram_tensor("w", [E, O, I * KK], BF16, kind="ExternalInput")
    at_d = nc.dram_tensor("alignT", [E, O, O], BF16, kind="ExternalInput")
    w1_d = nc.dram_tensor("w1T", [I, HID], F32, kind="ExternalInput")
    w2_d = nc.dram_tensor("w2T", [HID, E], F32, kind="ExternalInput")
    b2_d = nc.dram_tensor("b2", [E, 1], F32, kind="ExternalInput")
    bias_d = nc.dram_tensor("bias", [E, O], F32, kind="ExternalInput")
    out_d = nc.dram_tensor("out", [BL, O, H, W], F32, kind="ExternalOutput")

    const = ctx.enter_context(tc.tile_pool(name="const", bufs=1))
    wstream = ctx.enter_context(tc.tile_pool(name="wstream", bufs=6))
    xpool = ctx.enter_context(tc.tile_pool(name="x", bufs=1))
    aggp = ctx.enter_context(tc.tile_pool(name="agg", bufs=BL))
    tmpp = ctx.enter_context(tc.tile_pool(name="tmp", bufs=2))
    stagep = ctx.enter_context(tc.tile_pool(name="stage", bufs=4))
    t_psum = ctx.enter_context(tc.tile_pool(name="tps", bufs=3, space="PSUM"))
    s_psum = ctx.enter_context(tc.tile_pool(name="sps", bufs=1, space="PSUM"))
    c_psum = ctx.enter_context(tc.tile_pool(name="cps", bufs=4, space="PSUM"))
    NG = 3  # kk chunks per aggregation group (3 groups of 3)

    # ---- constants in ----
    at_sb = const.tile([128, E, 2, O], BF16)  # part = o_old % 128
    w1_sb = const.tile([128, 2, HID], F32)  # part = i % 128
    w2_sb = const.tile([HID, E], F32)
    b2_sb = const.tile([E, 1], F32)
    bias_sb = const.tile([E, O], F32)
    ones_sb = const.tile([1, 128], F32)
    walT = const.tile([128, E, KK, 2, O], BF16)  # part = i % 128 (per i-half)
    pooledT = const.tile([128, 2, BL], F32)  # part = i % 128
    h_sb = const.tile([HID, BL], F32)
    att_sb = const.tile([E, BL], F32)
    att_row = const.tile([1, BL * E], F32)
    att_bc = const.tile([128, BL, E], F32)
    aggb_sb = const.tile([128, 2, BL], F32)  # part = o % 128

    # ---- DMA schedule: x0 first (attention-b0 head), then weights, then x1-3
    x_sb = {}

    def dma_x(b):
        for ih in range(2):
            t = xpool.tile([128, HP, HP], BF16, tag=f"x{b}_{ih}")
            nc.sync.dma_start(out=t[:, :, :], in_=x_d[b, ih * 128 : (ih + 1) * 128, :, :])
            x_sb[(b, ih)] = t
            # pooled sum via tensor_scalar accum_out (4x bf16 mode); the main
            # output is an in-place x*1.0 no-op.
            nc.vector.tensor_scalar(
                out=t[:, :, :],
                in0=t[:, :, :],
                scalar1=1.0,
                scalar2=None,
                op0=mybir.AluOpType.mult,
                op1=mybir.AluOpType.add,
                accum_out=pooledT[:, ih, b : b + 1],
            )

    def attention(b):
        hp = s_psum.tile([HID, 1], F32, tag="sps")
        for ih in range(2):
            nc.tensor.matmul(
                hp[:, :],
                lhsT=w1_sb[:, ih, :],
                rhs=pooledT[:, ih, b : b + 1],
                start=(ih == 0),
                stop=(ih == 1),
            )
        nc.scalar.activation(
            h_sb[:, b : b + 1], hp[:, :], mybir.ActivationFunctionType.Relu
        )
        ap = s_psum.tile([E, 1], F32, tag="sps")
        nc.tensor.matmul(ap[:, :], lhsT=w2_sb[:, :], rhs=h_sb[:, b : b + 1])
        nc.scalar.activation(
            att_sb[:, b : b + 1],
            ap[:, :],
            mybir.ActivationFunctionType.Sigmoid,
            bias=b2_sb[:, 0:1],
        )
        # gather the 5 att values onto partition 0, then broadcast to all 128
        nc.sync.dma_start(out=att_row[0:1, b * E : (b + 1) * E], in_=att_sb[:, b : b + 1])
        bp = s_psum.tile([128, E], F32, tag="sps")
        nc.tensor.matmul(
            bp[:, :], lhsT=ones_sb[0:1, :], rhs=att_row[0:1, b * E : (b + 1) * E]
        )
        nc.vector.tensor_copy(out=att_bc[:, b, :], in_=bp[:, :])
        # aggregated bias agg_b[o, b] = sum_e att[e,b] * bias[e, o]
        for ot in range(2):
            gp = s_psum.tile([128, 1], F32, tag="sps")
            nc.tensor.matmul(
                gp[:, :],
                lhsT=bias_sb[:, ot * 128 : (ot + 1) * 128],
                rhs=att_sb[:, b : b + 1],
            )
            nc.vector.tensor_copy(out=aggb_sb[:, ot, b : b + 1], in_=gp[:, :])

    wt_all = {}
    for e in range(E):
        for oh in range(2):
            t = wstream.tile([128, I, KK], BF16, tag="wst")
            nc.sync.dma_start(out=t[:, :, :], in_=w_d[e, oh * 128 : (oh + 1) * 128, :])
            wt_all[(e, oh)] = t
            nc.sync.dma_start(
                out=at_sb[:, e, oh, :], in_=at_d[e, oh * 128 : (oh + 1) * 128, :]
            )
    for ih in range(2):
        nc.sync.dma_start(out=w1_sb[:, ih, :], in_=w1_d[ih * 128 : (ih + 1) * 128, :])
    nc.sync.dma_start(out=w2_sb[:, :], in_=w2_d[:, :])
    nc.sync.dma_start(out=b2_sb[:, :], in_=b2_d[:, :])
    nc.sync.dma_start(out=bias_sb[:, :], in_=bias_d[:, :])
    nc.vector.memset(ones_sb[:, :], 1.0)
    for b in range(BL):
        dma_x(b)

    # ---- align transform: walT[e][kk, ih][ii, o] = sum_o_old w[e,o_old,(i,kk)] alignT[e][o_old, o]
    # Full-bank PSUM groups: both ih halves of one (e,kk) in one [128,512] bank.
    # Evacuations alternate DVE/ACT so neither engine backpressures the PE.
    for e in range(E):
        for kk in range(KK):
            tp = t_psum.tile([128, 2, O], F32, tag="tps")
            for ih in range(2):
                for oh in range(2):
                    nc.tensor.matmul(
                        tp[:, ih, :],
                        lhsT=wt_all[(e, oh)][:, ih * 128 : (ih + 1) * 128, kk],
                        rhs=at_sb[:, e, oh, :],
                        start=(oh == 0),
                        stop=(oh == 1),
                    )
            if (e * KK + kk) % 2 == 0:
                nc.vector.tensor_copy(out=walT[:, e, kk, :, :], in_=tp[:, :, :])
            else:
                nc.scalar.activation(
                    walT[:, e, kk, :, :],
                    tp[:, :, :],
                    mybir.ActivationFunctionType.Copy,
                )

    for b in range(BL):
        attention(b)

    # ---- per-sample: aggregate weights (DVE, kk-chunked) then conv (PE) ----
    for b in range(BL):
        # scale+add tree: tensor_scalar (4x bf16) + tensor_tensor (2x bf16)
        # beats the 1x-mode fused scalar_tensor_tensor chain. Chunked by kk
        # group so the conv can start consuming before the full agg is done.
        aggs = []
        for g in range(NG):
            k0, k1 = g * 3, g * 3 + 3
            agg = aggp.tile([128, 3, 2, O], BF16, tag=f"agg{g}")
            nc.vector.tensor_scalar_mul(
                agg[:, :, :, :], walT[:, 0, k0:k1, :, :], att_bc[:, b, 0:1]
            )
            for e in range(1, E):
                tmp = tmpp.tile([128, 3, 2, O], BF16, tag="tmp")
                nc.vector.tensor_scalar_mul(
                    tmp[:, :, :, :], walT[:, e, k0:k1, :, :], att_bc[:, b, e : e + 1]
                )
                nc.vector.tensor_add(
                    out=agg[:, :, :, :], in0=agg[:, :, :, :], in1=tmp[:, :, :, :]
                )
            aggs.append(agg)

        for ot in range(2):
            for blk in range(NBLK):
                cp = c_psum.tile([128, RB, W], F32, tag="cps")
                r0 = blk * RB
                n_mm = 2 * KK
                k = 0
                for g in range(NG):
                    for kq in range(3):
                        kk = g * 3 + kq
                        di, dj = kk // 3 - 1, kk % 3 - 1
                        for ih in range(2):
                            nc.tensor.matmul(
                                cp[:, :, :],
                                lhsT=aggs[g][:, kq, ih, ot * 128 : (ot + 1) * 128],
                                rhs=x_sb[(b, ih)][
                                    :, r0 + di + 1 : r0 + di + 1 + RB, dj + 1 : dj + 1 + W
                                ],
                                start=(k == 0),
                                stop=(k == n_mm - 1),
                            )
                            k += 1
                st = stagep.tile([128, RB, W], F32, tag="stage")
                nc.scalar.activation(
                    st[:, :, :],
                    cp[:, :, :],
                    mybir.ActivationFunctionType.Identity,
                    bias=aggb_sb[:, ot, b : b + 1],
                )
                nc.sync.dma_start(
                    out=out_d[b, ot * 128 : (ot + 1) * 128, r0 : r0 + RB, :],
                    in_=st[:, :, :],
                )


def _build():
    nc = bacc.Bacc("TRN2", target_bir_lowering=False, debug=False, num_devices=N_CORES)
    with contextlib.ExitStack() as ctx:
        tc = ctx.enter_context(tile.TileContext(nc))
        _emit(nc, tc, ctx)
    nc.compile()
    return nc


def _get_nc():
    global _NC_CACHE
    if _NC_CACHE is None:
        _NC_CACHE = _build()
    return _NC_CACHE


def _run(trace=False, **inputs):
    x = np.asarray(inputs["x"], np.float32)
    weight = np.asarray(inputs["weight"], np.float32)
    bias = np.asarray(inputs["bias"], np.float32)
    align = np.asarray(inputs["align"], np.float32)
    w1 = np.asarray(inputs["attn_w1"], np.float32)
    w2 = np.asarray(inputs["attn_w2"], np.float32)
    b2 = np.asarray(inputs["attn_b2"], np.float32)

    xp = np.zeros((B, I, HP, HP), dtype=ml_dtypes.bfloat16)
    xp[:, :, 1 : 1 + H, 1 : 1 + W] = x
    w_bf = weight.reshape(E, O, I * KK).astype(ml_dtypes.bfloat16)
    alT = np.ascontiguousarray(align.transpose(0, 2, 1)).astype(ml_dtypes.bfloat16)
    w1T = np.ascontiguousarray((w1 / float(H * W)).T)
    w2T = np.ascontiguousarray(w2.T)
    b2c = np.ascontiguousarray(b2.reshape(E, 1))

    nc = _get_nc()
    in_maps = []
    for c in range(N_CORES):
        in_maps.append(
            {
                "x": xp[c * BL : (c + 1) * BL],
                "w": w_bf,
                "alignT": alT,
                "w1T": w1T,
                "w2T": w2T,
                "b2": b2c,
                "bias": bias,
            }
        )
    if trace:
        _install_ntff_hook()
    res = run_bass_kernel_spmd(
        nc, in_maps, core_ids=list(range(N_CORES)), trace=trace
    )
    out = np.concatenate([res.results[c]["out"] for c in range(N_CORES)], axis=0)
    return out, res


def kernel(**inputs):
    out, _ = _run(trace=False, **inputs)
    return out


def kernel_profiled(**inputs):
    out, res = _run(trace=True, **inputs)
    return out, res


# revision 17
# speedup vs baseline: 1.2197x; 1.0075x over previous
"""AdaAggLayer Trainium2 kernel.

Data-parallel over batch: 8 NeuronCores x 4 samples each. Per core:
  - attention (global avg pool -> 1x1 -> relu -> 1x1 -> sigmoid) on PE/ACT/DVE
  - align transform w_alT[e] = (align[e] @ w[e]) stored transposed [i,o] on PE
  - per-sample weight aggregation sum_e att[b,e]*w_alT[e] on DVE (bf16)
  - per-sample 3x3 conv as 9 shifted matmuls accumulating in PSUM (bf16)
  - bias epilogue fused into the PSUM->SBUF copy on ACT
No collectives: inputs are sharded/replicated host-side, outputs concatenated.
"""

import contextlib
import importlib.util
import os
import sys
import types

sys.path.insert(0, "/opt/trn_rl_repo")

import numpy as np
import ml_dtypes

import concourse.bass as bass
import concourse.mybir as mybir
import concourse.tile as tile
from concourse import bacc
from concourse.bass_utils import run_bass_kernel_spmd

N_CORES = 8
B, I, O, E, HID = 32, 256, 256, 5, 65
H = W = 56
HP = H + 2  # zero-padded spatial
BL = B // N_CORES  # samples per core
KK = 9  # 3x3 taps
NBLK = 7  # row blocks of 8 output rows
RB = 8  # rows per block
BF16 = mybir.dt.bfloat16
F32 = mybir.dt.float32

_NC_CACHE = None


def _install_ntff_hook():
    """Register the axon NTFF profiling hook (the image's antenv lacks it)."""
    if "antenv.axon_hooks" in sys.modules:
        return
    try:
        spec = importlib.util.spec_from_file_location(
            "trn_boot", "/root/.axon_site/trn_agent_boot/trn_boot.py"
        )
        tb = importlib.util.module_from_spec(spec)
        spec.loader.exec_module(tb)
        hook = tb._ntff_profile_via_ctypes("/opt/axon/libaxon_pjrt.so")
    except Exception:
        hook = None
    mod = types.ModuleType("antenv.axon_hooks")
    mod.get_axon_ntff_profile_hook = lambda: hook
    sys.modules["antenv.axon_hooks"] = mod


def _emit(nc, tc, ctx):
    x_d = nc.dram_tensor("x", [BL, I, HP, HP], BF16, kind="ExternalInput")
    w_d = nc.dram_tensor("w", [E, O, I * KK], BF16, kind="ExternalInput")
    at_d = nc.dram_tensor("alignT", [E, O, O], BF16, kind="ExternalInput")
    w1_d = nc.dram_tensor("w1T", [I, HID], F32, kind="ExternalInput")
    w2_d = nc.dram_tensor("w2T", [HID, E], F32, kind="ExternalInput")
    b2_d = nc.dram_tensor("b2", [E, 1], F32, kind="ExternalInput")
    bias_d = nc.dram_tensor("bias", [E, O], F32, kind="ExternalInput")
    out_d = nc.dram_tensor("out", [BL, O, H, W], F32, kind="ExternalOutput")

    const = ctx.enter_context(tc.tile_pool(name="const", bufs=1))
    wstream = ctx.enter_context(tc.tile_pool(name="wstream", bufs=6))
    xpool = ctx.enter_context(tc.tile_pool(name="x", bufs=1))
    aggp = ctx.enter_context(tc.tile_pool(name="agg", bufs=BL))
    tmpp = ctx.enter_context(tc.tile_pool(name="tmp", bufs=2))
    stagep = ctx.enter_context(tc.tile_pool(name="stage", bufs=4))
    t_psum = ctx.enter_context(tc.tile_pool(name="tps", bufs=3, space="PSUM"))
    s_psum = ctx.enter_context(tc.tile_pool(name="sps", bufs=1, space="PSUM"))
    c_psum = ctx.enter_context(tc.tile_pool(name="cps", bufs=4, space="PSUM"))
    NG = 3  # kk chunks per aggregation group (3 groups of 3)

    # ---- constants in ----
    at_sb = const.tile([128, E, 2, O], BF16)  # part = o_old % 128
    w1_sb = const.tile([128, 2, HID], F32)  # part = i % 128
    w2_sb = const.tile([HID, E], F32)
    b2_sb = const.tile([E, 1], F32)
    bias_sb = const.tile([E, O], F32)
    ones_sb = const.tile([1, 128], F32)
    walT = const.tile([128, E, KK, 2, O], BF16)  # part = i % 128 (per i-half)
    pooledT = const.tile([128, 2, BL], F32)  # part = i % 128
    h_sb = const.tile([HID, BL], F32)
    att_sb = const.tile([E, BL], F32)
    att_row = const.tile([1, BL * E], F32)
    att_bc = const.tile([128, BL, E], F32)
    aggb_sb = const.tile([128, 2, BL], F32)  # part = o % 128

    # ---- DMA schedule: x0 first (attention-b0 head), then weights, then x1-3
    x_sb = {}

    def dma_x(b):
        for ih in range(2):
            t = xpool.tile([128, HP, HP], BF16, tag=f"x{b}_{ih}")
            nc.sync.dma_start(out=t[:, :, :], in_=x_d[b, ih * 128 : (ih + 1) * 128, :, :])
            x_sb[(b, ih)] = t
            # pooled sum via tensor_scalar accum_out (4x bf16 mode); the main
            # output is an in-place x*1.0 no-op.
            nc.vector.tensor_scalar(
                out=t[:, :, :],
                in0=t[:, :, :],
                scalar1=1.0,
                scalar2=None,
                op0=mybir.AluOpType.mult,
                op1=mybir.AluOpType.add,
                accum_out=pooledT[:, ih, b : b + 1],
            )

    def attention(b):
        hp = s_psum.tile([HID, 1], F32, tag="sps")
        for ih in range(2):
            nc.tensor.matmul(
                hp[:, :],
                lhsT=w1_sb[:, ih, :],
                rhs=pooledT[:, ih, b : b + 1],
                start=(ih == 0),
                stop=(ih == 1),
            )
        nc.scalar.activation(
            h_sb[:, b : b + 1], hp[:, :], mybir.ActivationFunctionType.Relu
        )
        ap = s_psum.tile([E, 1], F32, tag="sps")
        nc.tensor.matmul(ap[:, :], lhsT=w2_sb[:, :], rhs=h_sb[:, b : b + 1])
        nc.scalar.activation(
            att_sb[:, b : b + 1],
            ap[:, :],
            mybir.ActivationFunctionType.Sigmoid,
            bias=b2_sb[:, 0:1],
        )
        # gather the 5 att values onto partition 0, then broadcast to all 128
        nc.sync.dma_start(out=att_row[0:1, b * E : (b + 1) * E], in_=att_sb[:, b : b + 1])
        bp = s_psum.tile([128, E], F32, tag="sps")
        nc.tensor.matmul(
            bp[:, :], lhsT=ones_sb[0:1, :], rhs=att_row[0:1, b * E : (b + 1) * E]
        )
        nc.vector.tensor_copy(out=att_bc[:, b, :], in_=bp[:, :])
        # aggregated bias agg_b[o, b] = sum_e att[e,b] * bias[e, o]
        for ot in range(2):
            gp = s_psum.tile([128, 1], F32, tag="sps")
            nc.tensor.matmul(
                gp[:, :],
                lhsT=bias_sb[:, ot * 128 : (ot + 1) * 128],
                rhs=att_sb[:, b : b + 1],
            )
            nc.vector.tensor_copy(out=aggb_sb[:, ot, b : b + 1], in_=gp[:, :])

    dma_x(0)
    wt_all = {}
    for e in range(E):
        for oh in range(2):
            t = wstream.tile([128, I, KK], BF16, tag="wst")
            nc.sync.dma_start(out=t[:, :, :], in_=w_d[e, oh * 128 : (oh + 1) * 128, :])
            wt_all[(e, oh)] = t
            nc.sync.dma_start(
                out=at_sb[:, e, oh, :], in_=at_d[e, oh * 128 : (oh + 1) * 128, :]
            )
    for ih in range(2):
        nc.sync.dma_start(out=w1_sb[:, ih, :], in_=w1_d[ih * 128 : (ih + 1) * 128, :])
    nc.sync.dma_start(out=w2_sb[:, :], in_=w2_d[:, :])
    nc.sync.dma_start(out=b2_sb[:, :], in_=b2_d[:, :])
    nc.sync.dma_start(out=bias_sb[:, :], in_=bias_d[:, :])
    nc.vector.memset(ones_sb[:, :], 1.0)
    for b in range(1, BL):
        dma_x(b)

    # ---- align transform: walT[e][kk, ih][ii, o] = sum_o_old w[e,o_old,(i,kk)] alignT[e][o_old, o]
    # Full-bank PSUM groups: both ih halves of one (e,kk) in one [128,512] bank.
    # Evacuations alternate DVE/ACT so neither engine backpressures the PE.
    # attention(0) is slotted mid-transform: its pooled input is ready by then
    # and conv0 needs att0 right after the transform finishes; b1-3 attention
    # goes after the transform (before conv0 in the PE FIFO, data all landed).
    for e in range(E):
        for kk in range(KK):
            tp = t_psum.tile([128, 2, O], F32, tag="tps")
            for ih in range(2):
                for oh in range(2):
                    nc.tensor.matmul(
                        tp[:, ih, :],
                        lhsT=wt_all[(e, oh)][:, ih * 128 : (ih + 1) * 128, kk],
                        rhs=at_sb[:, e, oh, :],
                        start=(oh == 0),
                        stop=(oh == 1),
                    )
            if (e * KK + kk) % 2 == 0:
                nc.vector.tensor_copy(out=walT[:, e, kk, :, :], in_=tp[:, :, :])
            else:
                nc.scalar.activation(
                    walT[:, e, kk, :, :],
                    tp[:, :, :],
                    mybir.ActivationFunctionType.Copy,
                )
        if e == 2:
            attention(0)

    for b in range(1, BL):
        attention(b)

    # ---- per-sample: aggregate weights (DVE, kk-chunked) then conv (PE) ----
    for b in range(BL):
        # scale+add tree: tensor_scalar (4x bf16) + tensor_tensor (2x bf16)
        # beats the 1x-mode fused scalar_tensor_tensor chain. Chunked by kk
        # group so the conv can start consuming before the full agg is done.
        aggs = []
        for g in range(NG):
            k0, k1 = g * 3, g * 3 + 3
            agg = aggp.tile([128, 3, 2, O], BF16, tag=f"agg{g}")
            nc.vector.tensor_scalar_mul(
                agg[:, :, :, :], walT[:, 0, k0:k1, :, :], att_bc[:, b, 0:1]
            )
            for e in range(1, E):
                tmp = tmpp.tile([128, 3, 2, O], BF16, tag="tmp")
                nc.vector.tensor_scalar_mul(
                    tmp[:, :, :, :], walT[:, e, k0:k1, :, :], att_bc[:, b, e : e + 1]
                )
                nc.vector.tensor_add(
                    out=agg[:, :, :, :], in0=agg[:, :, :, :], in1=tmp[:, :, :, :]
                )
            aggs.append(agg)

        for ot in range(2):
            for blk in range(NBLK):
                cp = c_psum.tile([128, RB, W], F32, tag="cps")
                r0 = blk * RB
                n_mm = 2 * KK
                k = 0
                for g in range(NG):
                    for kq in range(3):
                        kk = g * 3 + kq
                        di, dj = kk // 3 - 1, kk % 3 - 1
                        for ih in range(2):
                            nc.tensor.matmul(
                                cp[:, :, :],
                                lhsT=aggs[g][:, kq, ih, ot * 128 : (ot + 1) * 128],
                                rhs=x_sb[(b, ih)][
                                    :, r0 + di + 1 : r0 + di + 1 + RB, dj + 1 : dj + 1 + W
                                ],
                                start=(k == 0),
                                stop=(k == n_mm - 1),
                            )
                            k += 1
                st = stagep.tile([128, RB, W], F32, tag="stage")
                nc.scalar.activation(
                    st[:, :, :],
                    cp[:, :, :],
                    mybir.ActivationFunctionType.Identity,
                    bias=aggb_sb[:, ot, b : b + 1],
                )
                nc.sync.dma_start(
                    out=out_d[b, ot * 128 : (ot + 1) * 128, r0 : r0 + RB, :],
                    in_=st[:, :, :],
                )


def _build():
    nc = bacc.Bacc("TRN2", target_bir_lowering=False, debug=False, num_devices=N_CORES)
    with contextlib.ExitStack() as ctx:
        tc = ctx.enter_context(tile.TileContext(nc))
        _emit(nc, tc, ctx)
    nc.compile()
    return nc


def _get_nc():
    global _NC_CACHE
    if _NC_CACHE is None:
        _NC_CACHE = _build()
    return _NC_CACHE


def _run(trace=False, **inputs):
    x = np.asarray(inputs["x"], np.float32)
    weight = np.asarray(inputs["weight"], np.float32)
    bias = np.asarray(inputs["bias"], np.float32)
    align = np.asarray(inputs["align"], np.float32)
    w1 = np.asarray(inputs["attn_w1"], np.float32)
    w2 = np.asarray(inputs["attn_w2"], np.float32)
    b2 = np.asarray(inputs["attn_b2"], np.float32)

    xp = np.zeros((B, I, HP, HP), dtype=ml_dtypes.bfloat16)
    xp[:, :, 1 : 1 + H, 1 : 1 + W] = x
    w_bf = weight.reshape(E, O, I * KK).astype(ml_dtypes.bfloat16)
    alT = np.ascontiguousarray(align.transpose(0, 2, 1)).astype(ml_dtypes.bfloat16)
    w1T = np.ascontiguousarray((w1 / float(H * W)).T)
    w2T = np.ascontiguousarray(w2.T)
    b2c = np.ascontiguousarray(b2.reshape(E, 1))

    nc = _get_nc()
    in_maps = []
    for c in range(N_CORES):
        in_maps.append(
            {
                "x": xp[c * BL : (c + 1) * BL],
                "w": w_bf,
                "alignT": alT,
                "w1T": w1T,
                "w2T": w2T,
                "b2": b2c,
                "bias": bias,
            }
        )
    if trace:
        _install_ntff_hook()
    res = run_bass_kernel_spmd(
        nc, in_maps, core_ids=list(range(N_CORES)), trace=trace
    )
    out = np.concatenate([res.results[c]["out"] for c in range(N_CORES)], axis=0)
    return out, res


def kernel(**inputs):
    out, _ = _run(trace=False, **inputs)
    return out


def kernel_profiled(**inputs):
    out, res = _run(trace=True, **inputs)
    return out, res


# revision 20
# speedup vs baseline: 1.2225x; 1.0023x over previous
"""AdaAggLayer Trainium2 kernel.

Data-parallel over batch: 8 NeuronCores x 4 samples each. Per core:
  - attention (global avg pool -> 1x1 -> relu -> 1x1 -> sigmoid) on PE/ACT/DVE
  - align transform w_alT[e] = (align[e] @ w[e]) stored transposed [i,o] on PE
  - per-sample weight aggregation sum_e att[b,e]*w_alT[e] on DVE (bf16)
  - per-sample 3x3 conv as 9 shifted matmuls accumulating in PSUM (bf16)
  - bias epilogue fused into the PSUM->SBUF copy on ACT
No collectives: inputs are sharded/replicated host-side, outputs concatenated.
"""

import contextlib
import importlib.util
import os
import sys
import types

sys.path.insert(0, "/opt/trn_rl_repo")

import numpy as np
import ml_dtypes

import concourse.bass as bass
import concourse.mybir as mybir
import concourse.tile as tile
from concourse import bacc
from concourse.bass_utils import run_bass_kernel_spmd

N_CORES = 8
B, I, O, E, HID = 32, 256, 256, 5, 65
H = W = 56
HP = H + 2  # zero-padded spatial
BL = B // N_CORES  # samples per core
KK = 9  # 3x3 taps
NBLK = 7  # row blocks of 8 output rows
RB = 8  # rows per block
BF16 = mybir.dt.bfloat16
F32 = mybir.dt.float32

_NC_CACHE = None


def _install_ntff_hook():
    """Register the axon NTFF profiling hook (the image's antenv lacks it)."""
    if "antenv.axon_hooks" in sys.modules:
        return
    try:
        spec = importlib.util.spec_from_file_location(
            "trn_boot", "/root/.axon_site/trn_agent_boot/trn_boot.py"
        )
        tb = importlib.util.module_from_spec(spec)
        spec.loader.exec_module(tb)
        hook = tb._ntff_profile_via_ctypes("/opt/axon/libaxon_pjrt.so")
    except Exception:
        hook = None
    mod = types.ModuleType("antenv.axon_hooks")
    mod.get_axon_ntff_profile_hook = lambda: hook
    sys.modules["antenv.axon_hooks"] = mod


def _emit(nc, tc, ctx):
    x_d = nc.dram_tensor("x", [BL, I, HP, HP], BF16, kind="ExternalInput")
    w_d = nc.dram_tensor("w", [E, O, I * KK], BF16, kind="ExternalInput")
    at_d = nc.dram_tensor("alignT", [E, O, O], BF16, kind="ExternalInput")
    w1_d = nc.dram_tensor("w1T", [I, HID], F32, kind="ExternalInput")
    w2_d = nc.dram_tensor("w2T", [HID, E], F32, kind="ExternalInput")
    b2_d = nc.dram_tensor("b2", [E, 1], F32, kind="ExternalInput")
    bias_d = nc.dram_tensor("bias", [E, O], F32, kind="ExternalInput")
    out_d = nc.dram_tensor("out", [BL, O, H, W], F32, kind="ExternalOutput")

    const = ctx.enter_context(tc.tile_pool(name="const", bufs=1))
    wstream = ctx.enter_context(tc.tile_pool(name="wstream", bufs=6))
    xpool = ctx.enter_context(tc.tile_pool(name="x", bufs=1))
    aggp = ctx.enter_context(tc.tile_pool(name="agg", bufs=BL))
    tmpp = ctx.enter_context(tc.tile_pool(name="tmp", bufs=2))
    stagep = ctx.enter_context(tc.tile_pool(name="stage", bufs=4))
    t_psum = ctx.enter_context(tc.tile_pool(name="tps", bufs=3, space="PSUM"))
    s_psum = ctx.enter_context(tc.tile_pool(name="sps", bufs=1, space="PSUM"))
    c_psum = ctx.enter_context(tc.tile_pool(name="cps", bufs=4, space="PSUM"))
    NG = 3  # kk chunks per aggregation group (3 groups of 3)

    # ---- constants in ----
    at_sb = const.tile([128, E, 2, O], BF16)  # part = o_old % 128
    w1_sb = const.tile([128, 2, HID], F32)  # part = i % 128
    w2_sb = const.tile([HID, E], F32)
    b2_sb = const.tile([E, 1], F32)
    bias_sb = const.tile([E, O], F32)
    ones_sb = const.tile([1, 128], F32)
    walT = const.tile([128, E, KK, 2, O], BF16)  # part = i % 128 (per i-half)
    pooledT = const.tile([128, 2, BL], F32)  # part = i % 128
    h_sb = const.tile([HID, BL], F32)
    att_sb = const.tile([E, BL], F32)
    att_row = const.tile([1, BL * E], F32)
    att_bc = const.tile([128, BL, E], F32)
    aggb_sb = const.tile([128, 2, BL], F32)  # part = o % 128

    x_sb = {}

    def dma_x(b):
        for ih in range(2):
            t = xpool.tile([128, HP, HP], BF16, tag=f"x{b}_{ih}")
            nc.sync.dma_start(out=t[:, :, :], in_=x_d[b, ih * 128 : (ih + 1) * 128, :, :])
            x_sb[(b, ih)] = t

    def attention(b):
        # pooled sum via tensor_scalar accum_out (4x bf16 mode); the main
        # output is an in-place x*1.0 no-op. Emitted here (not at the DMA)
        # so the DVE stream doesn't head-of-line block on the x DMA.
        for ih in range(2):
            t = x_sb[(b, ih)]
            nc.vector.tensor_scalar(
                out=t[:, :, :],
                in0=t[:, :, :],
                scalar1=1.0,
                scalar2=None,
                op0=mybir.AluOpType.mult,
                op1=mybir.AluOpType.add,
                accum_out=pooledT[:, ih, b : b + 1],
            )
        hp = s_psum.tile([HID, 1], F32, tag="sps")
        for ih in range(2):
            nc.tensor.matmul(
                hp[:, :],
                lhsT=w1_sb[:, ih, :],
                rhs=pooledT[:, ih, b : b + 1],
                start=(ih == 0),
                stop=(ih == 1),
            )
        nc.scalar.activation(
            h_sb[:, b : b + 1], hp[:, :], mybir.ActivationFunctionType.Relu
        )
        ap = s_psum.tile([E, 1], F32, tag="sps")
        nc.tensor.matmul(ap[:, :], lhsT=w2_sb[:, :], rhs=h_sb[:, b : b + 1])
        nc.scalar.activation(
            att_sb[:, b : b + 1],
            ap[:, :],
            mybir.ActivationFunctionType.Sigmoid,
            bias=b2_sb[:, 0:1],
        )
        # gather the 5 att values onto partition 0, then broadcast to all 128
        nc.sync.dma_start(out=att_row[0:1, b * E : (b + 1) * E], in_=att_sb[:, b : b + 1])
        bp = s_psum.tile([128, E], F32, tag="sps")
        nc.tensor.matmul(
            bp[:, :], lhsT=ones_sb[0:1, :], rhs=att_row[0:1, b * E : (b + 1) * E]
        )
        nc.vector.tensor_copy(out=att_bc[:, b, :], in_=bp[:, :])
        # aggregated bias agg_b[o, b] = sum_e att[e,b] * bias[e, o]
        for ot in range(2):
            gp = s_psum.tile([128, 1], F32, tag="sps")
            nc.tensor.matmul(
                gp[:, :],
                lhsT=bias_sb[:, ot * 128 : (ot + 1) * 128],
                rhs=att_sb[:, b : b + 1],
            )
            nc.vector.tensor_copy(out=aggb_sb[:, ot, b : b + 1], in_=gp[:, :])

    wt_all = {}
    for e in range(E):
        for oh in range(2):
            t = wstream.tile([128, I, KK], BF16, tag="wst")
            nc.sync.dma_start(out=t[:, :, :], in_=w_d[e, oh * 128 : (oh + 1) * 128, :])
            wt_all[(e, oh)] = t
            nc.sync.dma_start(
                out=at_sb[:, e, oh, :], in_=at_d[e, oh * 128 : (oh + 1) * 128, :]
            )
    for ih in range(2):
        nc.sync.dma_start(out=w1_sb[:, ih, :], in_=w1_d[ih * 128 : (ih + 1) * 128, :])
    nc.sync.dma_start(out=w2_sb[:, :], in_=w2_d[:, :])
    nc.sync.dma_start(out=b2_sb[:, :], in_=b2_d[:, :])
    nc.sync.dma_start(out=bias_sb[:, :], in_=bias_d[:, :])
    nc.vector.memset(ones_sb[:, :], 1.0)
    for b in range(BL):
        dma_x(b)

    # ---- align transform: walT[e][kk, ih][ii, o] = sum_o_old w[e,o_old,(i,kk)] alignT[e][o_old, o]
    # Full-bank PSUM groups: both ih halves of one (e,kk) in one [128,512] bank.
    # Evacuations alternate DVE/ACT so neither engine backpressures the PE.
    # attention(0) is slotted mid-transform: its pooled input is ready by then
    # and conv0 needs att0 right after the transform finishes. attention(b>0)
    # is slotted inside conv(b-1)'s matmul stream (see conv loop) so it never
    # gates earlier convs through the PE FIFO.
    for e in range(E):
        for kk in range(KK):
            tp = t_psum.tile([128, 2, O], F32, tag="tps")
            for ih in range(2):
                for oh in range(2):
                    nc.tensor.matmul(
                        tp[:, ih, :],
                        lhsT=wt_all[(e, oh)][:, ih * 128 : (ih + 1) * 128, kk],
                        rhs=at_sb[:, e, oh, :],
                        start=(oh == 0),
                        stop=(oh == 1),
                    )
            if (e * KK + kk) % 2 == 0:
                nc.vector.tensor_copy(out=walT[:, e, kk, :, :], in_=tp[:, :, :])
            else:
                nc.scalar.activation(
                    walT[:, e, kk, :, :],
                    tp[:, :, :],
                    mybir.ActivationFunctionType.Copy,
                )
        if e == 3:
            attention(0)

    # ---- per-sample: aggregate weights (DVE, kk-chunked) then conv (PE) ----
    for b in range(BL):
        # scale+add tree: tensor_scalar (4x bf16) + tensor_tensor (2x bf16)
        # beats the 1x-mode fused scalar_tensor_tensor chain. Chunked by kk
        # group so the conv can start consuming before the full agg is done.
        aggs = []
        for g in range(NG):
            k0, k1 = g * 3, g * 3 + 3
            agg = aggp.tile([128, 3, 2, O], BF16, tag=f"agg{g}")
            nc.vector.tensor_scalar_mul(
                agg[:, :, :, :], walT[:, 0, k0:k1, :, :], att_bc[:, b, 0:1]
            )
            for e in range(1, E):
                tmp = tmpp.tile([128, 3, 2, O], BF16, tag="tmp")
                nc.vector.tensor_scalar_mul(
                    tmp[:, :, :, :], walT[:, e, k0:k1, :, :], att_bc[:, b, e : e + 1]
                )
                nc.vector.tensor_add(
                    out=agg[:, :, :, :], in0=agg[:, :, :, :], in1=tmp[:, :, :, :]
                )
            aggs.append(agg)

        for ot in range(2):
            for blk in range(NBLK):
                cp = c_psum.tile([128, RB, W], F32, tag="cps")
                r0 = blk * RB
                n_mm = 2 * KK
                k = 0
                for g in range(NG):
                    for kq in range(3):
                        kk = g * 3 + kq
                        di, dj = kk // 3 - 1, kk % 3 - 1
                        for ih in range(2):
                            nc.tensor.matmul(
                                cp[:, :, :],
                                lhsT=aggs[g][:, kq, ih, ot * 128 : (ot + 1) * 128],
                                rhs=x_sb[(b, ih)][
                                    :, r0 + di + 1 : r0 + di + 1 + RB, dj + 1 : dj + 1 + W
                                ],
                                start=(k == 0),
                                stop=(k == n_mm - 1),
                            )
                            k += 1
                st = stagep.tile([128, RB, W], F32, tag="stage")
                nc.scalar.activation(
                    st[:, :, :],
                    cp[:, :, :],
                    mybir.ActivationFunctionType.Identity,
                    bias=aggb_sb[:, ot, b : b + 1],
                )
                nc.sync.dma_start(
                    out=out_d[b, ot * 128 : (ot + 1) * 128, r0 : r0 + RB, :],
                    in_=st[:, :, :],
                )
                # next sample's attention rides inside this conv stream: its
                # x has landed by now, and it must finish before this conv
                # ends so agg(b+1) (DVE) overlaps the conv tail.
                if ot == 0 and blk == 2 and b + 1 < BL:
                    attention(b + 1)


def _build():
    nc = bacc.Bacc("TRN2", target_bir_lowering=False, debug=False, num_devices=N_CORES)
    with contextlib.ExitStack() as ctx:
        tc = ctx.enter_context(tile.TileContext(nc))
        _emit(nc, tc, ctx)
    nc.compile()
    return nc


def _get_nc():
    global _NC_CACHE
    if _NC_CACHE is None:
        _NC_CACHE = _build()
    return _NC_CACHE


def _run(trace=False, **inputs):
    x = np.asarray(inputs["x"], np.float32)
    weight = np.asarray(inputs["weight"], np.float32)
    bias = np.asarray(inputs["bias"], np.float32)
    align = np.asarray(inputs["align"], np.float32)
    w1 = np.asarray(inputs["attn_w1"], np.float32)
    w2 = np.asarray(inputs["attn_w2"], np.float32)
    b2 = np.asarray(inputs["attn_b2"], np.float32)

    xp = np.zeros((B, I, HP, HP), dtype=ml_dtypes.bfloat16)
    xp[:, :, 1 : 1 + H, 1 : 1 + W] = x
    w_bf = weight.reshape(E, O, I * KK).astype(ml_dtypes.bfloat16)
    alT = np.ascontiguousarray(align.transpose(0, 2, 1)).astype(ml_dtypes.bfloat16)
    w1T = np.ascontiguousarray((w1 / float(H * W)).T)
    w2T = np.ascontiguousarray(w2.T)
    b2c = np.ascontiguousarray(b2.reshape(E, 1))

    nc = _get_nc()
    in_maps = []
    for c in range(N_CORES):
        in_maps.append(
            {
                "x": xp[c * BL : (c + 1) * BL],
                "w": w_bf,
                "alignT": alT,
                "w1T": w1T,
                "w2T": w2T,
                "b2": b2c,
                "bias": bias,
            }
        )
    if trace:
        _install_ntff_hook()
    res = run_bass_kernel_spmd(
        nc, in_maps, core_ids=list(range(N_CORES)), trace=trace
    )
    out = np.concatenate([res.results[c]["out"] for c in range(N_CORES)], axis=0)
    return out, res


def kernel(**inputs):
    out, _ = _run(trace=False, **inputs)
    return out


def kernel_profiled(**inputs):
    out, res = _run(trace=True, **inputs)
    return out, res
